# revision 18
# baseline (speedup 1.0000x reference)
"""Trainium2 Bass kernel for nn_BlurLayer (B=128, 224x224x3, per-sample
rotated-line motion blur, SAME depthwise conv).

Self-contained: kernel(**inputs) -> np.ndarray. Shards the batch over 8
NeuronCores (pure data parallel: 16 samples per core), compiles + runs one
SPMD Bass program via concourse.bass_utils.run_bass_kernel_spmd, gathers
the full output.

Method: the rotated blur kernel's nonzero taps all equal 1/size and form a
digitized line. Per sample we pick the basis (identity / transpose /
unit shear, applied to the kernel) that minimizes PE streaming cost: taps
grouped by (sheared) kernel column give banded 0/1 weight matrices
contracted over image rows on the PE. The image is split into two fp8e4m3
planes (hi = fp8(x), lo = fp8(x - hi)) so each matmul runs in DoubleRow
perf mode (2 fp8 MACs/PE/cycle). Horizontal alignment and the shear are
baked into the host-side blob layout (a shear is linear in the row index),
so all device access patterns are static; sheared outputs are written to
DRAM in sheared form and unsheared on the host. Each slot's image windows
+ weight table arrive in one DMA; a post-compile pass drops back-to-back
reloads of identical PE weights; 1/size scaling happens in the PSUM->SBUF
copies, split between the Scalar and Vector engines.
"""

import math

import numpy as np
import ml_dtypes

MAXK = 32
H = W = 224
C = 3
WC = W * C                  # 672
PAD_LO = (MAXK - 1) // 2    # 15
PIMG_PAD = 480              # left zero margin (elems) of padded image row
PIMG_W = PIMG_PAD + WC + 600
WEDGE = 3 * 111             # extra sheared-output cols per 112-row block

FP8 = ml_dtypes.float8_e4m3


def dedupe_ldweights(nc):
    """Replace an InstLdweights whose weights AP is identical to the
    immediately-preceding one (per block) with an InstNoOp carrying its
    sync_info: the PE array still holds those weights, so the reload is
    pure Tensor-queue overhead (~158ns each)."""
    import concourse.mybir as mybir
    n = 0
    for fn in nc.m.functions:
        for blk in fn.blocks:
            prev_key = None
            new_insts = []
            for inst in blk.instructions:
                if isinstance(inst, mybir.InstLdweights):
                    key = (repr(inst.ins), repr(getattr(inst, "perf_mode", None)),
                           repr(getattr(inst, "is_transpose", None)))
                    if key == prev_key:
                        n += 1
                        inst = mybir.InstNoOp(
                            name=f"{inst.name}-ldwdedup",
                            engine=inst.engine,
                            ins=[], outs=[],
                            sync_info=inst.sync_info,
                        )
                    else:
                        prev_key = key
                new_insts.append(inst)
            blk.instructions = new_insts
    return n


# ---------------------------------------------------------------- host math
def rotate_nearest_np(img, rad):
    K = img.shape[0]
    cos, sin = np.cos(rad), np.sin(rad)
    coords = np.arange(K, dtype=np.float32)
    yy, xx = np.meshgrid(coords, coords, indexing="ij")
    e = np.float32(K - 1)
    x_off = (e - (cos * e - sin * e)) * 0.5
    y_off = (e - (sin * e + cos * e)) * 0.5
    sx = cos * xx - sin * yy + x_off
    sy = sin * xx + cos * yy + y_off
    ix = np.round(sx).astype(np.int32)
    iy = np.round(sy).astype(np.int32)
    valid = (ix >= 0) & (ix < K) & (iy >= 0) & (iy < K)
    g = img[np.clip(iy, 0, K - 1), np.clip(ix, 0, K - 1)]
    return np.where(valid, g, np.float32(0.0))


def _col_groups(ker):
    """Group nonzero taps of `ker` by column -> [(kx, klo, khi)], splitting
    any non-contiguous run."""
    ys, xs = np.nonzero(ker)
    groups = []
    for kx in np.unique(xs):
        run = np.sort(ys[xs == kx])
        start = prev = int(run[0])
        for v in run[1:]:
            v = int(v)
            if v == prev + 1:
                prev = v
            else:
                groups.append((int(kx), start, prev))
                start = prev = v
        groups.append((int(kx), start, prev))
    return groups


def _span(groups):
    if not groups:
        return 1
    kxs = [t[0] for t in groups]
    return max(kxs) - min(kxs) + 1


def shear_ker(ker):
    """ker'[ky, q] with q = kx - ky + 32 (unit shear; line angles in
    [0,90) have kx non-decreasing in ky, so this shrinks diagonal lines)."""
    K = ker.shape[0]
    wide = np.zeros((K, 2 * K + 1), ker.dtype)
    for ky in range(K):
        wide[ky, 32 - ky:32 - ky + K] = ker[ky]
    return wide


def sample_plan(tbl_ch0, amt_b, ang_b):
    """-> (scale, groups, transposed, sigma). groups are column-groups of
    the transformed kernel (column index q; true kx = q - 32*sigma +
    sigma*ky); basis minimizes span * streamed width."""
    rad = np.float32(ang_b * math.pi / 180.0)
    ker = rotate_nearest_np(tbl_ch0[amt_b], rad)
    ys, xs = np.nonzero(ker)
    if len(ys) == 0:
        return np.float32(0.0), [], False, 0
    scale = float(ker[ys[0], xs[0]])
    best = None
    for tr in (False, True):
        km = ker.T if tr else ker
        for sg in (0, 1):
            g = _col_groups(shear_ker(km) if sg else km)
            cost = _span(g) * (WC + WEDGE * sg)
            if best is None or cost < best[0]:
                best = (cost, g, tr, sg)
    return np.float32(scale), best[1], best[2], best[3]


def band_matrices(klo, khi):
    """w0 [128,112]: img rows 0..127 x out rows 0..111 (band r-y in
    [klo-15, khi-15]); w1: img rows 96..223 x out rows 112..223 (tile row
    r = img row 96+r, band r-y in [klo+1, khi+1]). Band clipping at the
    partition edges implements the vertical SAME padding."""
    r = np.arange(128)[:, None]
    y = np.arange(112)[None, :]
    d = r - y
    w0 = ((d >= klo - PAD_LO) & (d <= khi - PAD_LO)).astype(np.float32)
    w1 = ((d >= klo + 1) & (d <= khi + 1)).astype(np.float32)
    return w0, w1


def prepare_host(x, kernels_table, amt, angles, n_cores=8):
    B = x.shape[0]
    assert B % n_cores == 0
    slots = B // n_cores
    tbl_ch0 = np.ascontiguousarray(kernels_table[:, :, :, 0])

    scales = np.zeros(B, np.float32)
    groups = []
    transposed = np.zeros(B, bool)
    sigmas = np.zeros(B, np.int64)
    spans = np.zeros(B, np.int64)
    for b in range(B):
        s, g, tr, sg = sample_plan(tbl_ch0, int(amt[b]), int(angles[b]))
        scales[b] = s
        groups.append(g)
        transposed[b] = tr
        sigmas[b] = sg
        spans[b] = _span(g)

    # sigma must be uniform within a slot (SPMD). Round the sheared class
    # to a multiple of n_cores by moving the samples with the smallest
    # cost difference to the other basis, then balance on span per class.
    cost1 = np.array([spans[b] * (WC + WEDGE * sigmas[b]) for b in range(B)])
    cls = sigmas.copy()
    n1 = int((cls == 1).sum())
    rem = n1 % n_cores
    if rem:
        # re-plan candidates on the opposite basis and compute the penalty
        cand = np.where(cls == 1)[0]
        pen = []
        for b in cand:
            km = tbl_ch0[int(amt[b])]
            rad = np.float32(int(angles[b]) * math.pi / 180.0)
            ker = rotate_nearest_np(km, rad)
            g0 = _col_groups(ker)
            gt = _col_groups(ker.T)
            c0 = min(_span(g0) * WC, _span(gt) * WC)
            pen.append((c0 - cost1[b], b))
        pen.sort()
        for _, b in pen[:rem]:
            ker = rotate_nearest_np(tbl_ch0[int(amt[b])],
                                    np.float32(int(angles[b]) * math.pi / 180.0))
            g0 = _col_groups(ker)
            gt = _col_groups(ker.T)
            if _span(gt) < _span(g0):
                groups[b], transposed[b] = gt, True
            else:
                groups[b], transposed[b] = g0, False
            sigmas[b] = 0
            cls[b] = 0
            spans[b] = _span(groups[b])

    asg_rows = []
    row_sigma = []
    for sg in (0, 1):
        idx = np.where(cls == sg)[0]
        if len(idx) == 0:
            continue
        assert len(idx) % n_cores == 0
        order = idx[np.argsort(-spans[idx], kind="stable")]
        rows = order.reshape(len(idx) // n_cores, n_cores)
        for r in rows:
            asg_rows.append(r)
            row_sigma.append(sg)
    asg = np.stack(asg_rows)
    row_sigma = np.array(row_sigma)
    assert asg.shape == (slots, n_cores)

    # schedule: lightest slot first (its input DMA completes fastest, so
    # the PE starts early), a light slot last (small tail), heavy middle.
    slot_cost = np.array([max(1, spans[asg[j]].max()) * (WC + WEDGE * row_sigma[j])
                          for j in range(slots)])
    order = np.argsort(-slot_cost, kind="stable")   # heavy .. light
    sched = np.concatenate([[order[-2]], order[:-2], [order[-1]]])
    asg = asg[sched]
    row_sigma = row_sigma[sched]

    gmax = np.array([max(1, spans[asg[j]].max()) for j in range(slots)])
    wout = WC + WEDGE * row_sigma                    # result width per row block
    wprime = 3 * gmax + wout                         # moving window width
    wprime = ((wprime + 7) // 8) * 8
    blobw = 4 * wprime + 224 * gmax                  # 4 image planes + wt table
    col_base = np.concatenate([[0], np.cumsum(blobw)])[:-1]
    totbw = int(blobw.sum())
    out_base = np.concatenate([[0], np.cumsum(2 * 112 * wout)])[:-1]
    totout = int((2 * 112 * wout).sum())

    # fp8 hi/lo planes of the full batch
    x8hi = x.astype(FP8)
    xlo = x - x8hi.astype(np.float32)
    x8lo = xlo.astype(FP8)

    in_maps = []
    mapping = np.zeros((n_cores, slots), np.int64)
    for c in range(n_cores):
        blob = np.zeros((128, totbw), FP8)
        scl = np.zeros((128, slots), np.float32)
        for j in range(slots):
            b = int(asg[j, c])
            G = int(gmax[j])
            sg = int(row_sigma[j])
            Wp = int(wprime[j])
            base = int(col_base[j])
            mapping[c, j] = b
            scl[:, j] = scales[b]

            if transposed[b]:
                hi = np.ascontiguousarray(x8hi[b].transpose(1, 0, 2)).reshape(H, WC)
                lo = np.ascontiguousarray(x8lo[b].transpose(1, 0, 2)).reshape(H, WC)
            else:
                hi = x8hi[b].reshape(H, WC)
                lo = x8lo[b].reshape(H, WC)
            phi = np.zeros((H, PIMG_W), FP8)
            plo = np.zeros((H, PIMG_W), FP8)
            phi[:, PIMG_PAD:PIMG_PAD + WC] = hi
            plo[:, PIMG_PAD:PIMG_PAD + WC] = lo

            # group code cols q: true kx = q - 32*sg + sg*ky
            bk = min(t[0] for t in groups[b]) if groups[b] else 0
            # window row p of block hb covers pimg cols
            #   V0 + 3*sg*p + [0, Wp); out tile col u (psum col) holds
            #   out[R+r, w + 3*sg*r + u] with w = -WEDGE*sg.
            # matching: rhs col u' = u + 3*(q - bk) reads tap (q, ky) when
            #   V0 = PIMG_PAD - 45 + w + 3*(bk - 32*sg) + 3*sg*(S + 15 - R)
            for hb, (R, S) in enumerate(((0, 0), (112, 96))):
                V0 = (PIMG_PAD - 45 - WEDGE * sg + 3 * (bk - 32 * sg)
                      + 3 * sg * (S + 15 - R))
                assert 0 <= V0 and V0 + 3 * sg * 127 + Wp <= PIMG_W, \
                    (V0, sg, bk, Wp)
                rows = np.arange(128)
                cols = V0 + 3 * sg * rows
                src_rows = S + rows
                for pl, pimg in enumerate((phi, plo)):
                    dst = base + (2 * hb + pl) * Wp
                    win = np.zeros((128, Wp), FP8)
                    for p in range(128):
                        sr = src_rows[p]
                        if 0 <= sr < H:
                            win[p] = pimg[sr, cols[p]:cols[p] + Wp]
                    blob[:, dst:dst + Wp] = win

            # weight table: [G, 2(hb), 112] fp8, code col q = bk + g
            wtb = base + 4 * Wp
            wcols = np.zeros((128, G, 2, 112), np.float32)
            for q, klo, khi in groups[b]:
                i = q - bk
                assert 0 <= i < G, (b, q, bk, G)
                w0, w1 = band_matrices(klo, khi)
                wcols[:, i, 0, :] += w0
                wcols[:, i, 1, :] += w1
            blob[:, wtb:wtb + 224 * G] = np.ascontiguousarray(wcols).reshape(128, 224 * G).astype(FP8)
        in_maps.append({"ximg": blob, "scl": scl})

    meta = {
        "slots": slots,
        "gmax": [int(v) for v in gmax],
        "sigma": [int(v) for v in row_sigma],
        "wout": [int(v) for v in wout],
        "wprime": [int(v) for v in wprime],
        "blobw": [int(v) for v in blobw],
        "col_base": [int(v) for v in col_base],
        "out_base": [int(v) for v in out_base],
        "totbw": totbw,
        "totout": totout,
        "mapping": mapping,
        "transposed": transposed,
    }
    return meta, in_maps


def _chunks(wout):
    """Split a result width into <=512-col PSUM chunks (2 per row block)."""
    half = (wout + 1) // 2
    assert half <= 512
    return [(0, half), (half, wout - half)]


# ---------------------------------------------------------------- device IR
def build_program(meta):
    import concourse.bacc as bacc
    import concourse.mybir as mybir
    from concourse.tile import TileContext
    from bass_rust import VecI64Pair

    fp8 = mybir.dt.float8e4
    slots = meta["slots"]

    nc = bacc.Bacc("TRN2")
    ximg = nc.dram_tensor("ximg", [128, meta["totbw"]], fp8, kind="ExternalInput")
    scl = nc.dram_tensor("scl", [128, slots], mybir.dt.float32,
                         kind="ExternalInput")
    out = nc.dram_tensor("out", [1, meta["totout"]], mybir.dt.float16,
                         kind="ExternalOutput")

    def strided(tile, dims, offset):
        ap = tile[:, 0:1].copy()
        ap.ap = VecI64Pair(dims)
        ap.offset = offset
        return ap

    with TileContext(nc) as tc:
        with tc.tile_pool(name="const", bufs=1) as cpool, \
             tc.tile_pool(name="img", bufs=4) as ipool, \
             tc.tile_pool(name="res", bufs=4) as rpool, \
             tc.tile_pool(name="ps0", bufs=2, space="PSUM") as pp00, \
             tc.tile_pool(name="ps1", bufs=2, space="PSUM") as pp01, \
             tc.tile_pool(name="ps2", bufs=2, space="PSUM") as pp10, \
             tc.tile_pool(name="ps3", bufs=2, space="PSUM") as pp11:
            st = cpool.tile([128, slots], mybir.dt.float32)
            nc.scalar.dma_start(out=st, in_=scl[:, :])

            pools = [[pp00, pp01], [pp10, pp11]]
            for j in range(slots):
                G = meta["gmax"][j]
                WO = meta["wout"][j]
                Wp = meta["wprime"][j]
                BW = meta["blobw"][j]
                base = meta["col_base"][j]
                obase = meta["out_base"][j]
                ch = _chunks(WO)
                blob = ipool.tile([128, BW], fp8, tag="blob", name="blob")
                wtb = 4 * Wp
                nc.sync.dma_start(out=blob, in_=ximg[:, base:base + BW])

                psums = [[pools[hb][wh].tile([112, ch[wh][1]], mybir.dt.float32,
                                             tag=f"ps{hb}{wh}", name=f"ps{hb}{wh}")
                          for wh in (0, 1)] for hb in (0, 1)]
                rt = rpool.tile([112, 2 * WO], mybir.dt.float16, tag="rt",
                                name="rt")
                sc = st[0:112, j:j + 1]
                for hb in (0, 1):
                    for g in range(G):
                        # same band matrix for both fp8 planes (hi, lo)
                        lhs = strided(blob, [[BW, 128], [0, 2], [1, 112]],
                                      wtb + 224 * g + 112 * hb)
                        for wh in (0, 1):
                            # planes (hi, lo) of window hb at column shift 3g
                            rhs = strided(
                                blob, [[BW, 128], [Wp, 2], [1, ch[wh][1]]],
                                2 * hb * Wp + 3 * g + ch[wh][0])
                            nc.tensor.matmul(
                                psums[hb][wh], lhsT=lhs, rhs=rhs,
                                start=(g == 0), stop=(g == G - 1),
                                perf_mode=mybir.MatmulPerfMode.DoubleRow)
                    # drain this row block while the other still computes
                    dst0 = rt[:, hb * WO + ch[0][0]:hb * WO + ch[0][0] + ch[0][1]]
                    dst1 = rt[:, hb * WO + ch[1][0]:hb * WO + ch[1][0] + ch[1][1]]
                    nc.scalar.activation(
                        out=dst0, in_=psums[hb][0],
                        func=mybir.ActivationFunctionType.Copy, scale=sc)
                    nc.vector.tensor_scalar_mul(out=dst1, in0=psums[hb][1],
                                                scalar1=sc)
                    src = rt[:, hb * WO:hb * WO + WO]
                    dst = out[0, 0:1].copy()
                    dst.ap = VecI64Pair([[WO, 112], [1, WO]])
                    dst.offset = obase + hb * 112 * WO
                    nc.gpsimd.dma_start(out=dst, in_=src)
    return nc


def run_cores(meta, in_maps, trace=False):
    from concourse.bass_utils import run_bass_kernel_spmd

    nc = build_program(meta)
    nc.compile()
    dedupe_ldweights(nc)
    res = run_bass_kernel_spmd(nc, in_maps, core_ids=list(range(len(in_maps))),
                               trace=trace)
    return res


def unshard(meta, results):
    B = meta["mapping"].size
    out = np.zeros((B, H, W, C), np.float32)
    for c, r in enumerate(results):
        o = np.asarray(r["out"], np.float32).reshape(-1)
        for j in range(meta["slots"]):
            b = meta["mapping"][c, j]
            WO = meta["wout"][j]
            sg = meta["sigma"][j]
            t = o[meta["out_base"][j]:meta["out_base"][j] + 2 * 112 * WO]
            t = t.reshape(2, 112, WO)
            img = np.zeros((H, WC), np.float32)
            if sg == 0:
                img[0:112] = t[0, :, 0:WC]
                img[112:224] = t[1, :, 0:WC]
            else:
                for r_ in range(112):
                    u = 3 * (111 - r_)
                    img[r_] = t[0, r_, u:u + WC]
                    img[112 + r_] = t[1, r_, u:u + WC]
            img = img.reshape(H, W, C)
            if meta["transposed"][b]:
                img = img.transpose(1, 0, 2)
            out[b] = img
    return out


def kernel(x, kernels_table, amt, angles):
    x = np.asarray(x, np.float32)
    kernels_table = np.asarray(kernels_table, np.float32)
    amt = np.asarray(amt)
    angles = np.asarray(angles)
    meta, in_maps = prepare_host(x, kernels_table, amt, angles)
    res = run_cores(meta, in_maps)
    return unshard(meta, res.results)


# revision 19
# speedup vs baseline: 1.0066x; 1.0066x over previous
"""Trainium2 Bass kernel for nn_BlurLayer (B=128, 224x224x3, per-sample
rotated-line motion blur, SAME depthwise conv).

Self-contained: kernel(**inputs) -> np.ndarray. Shards the batch over 8
NeuronCores (pure data parallel: 16 samples per core), compiles + runs one
SPMD Bass program via concourse.bass_utils.run_bass_kernel_spmd, gathers
the full output.

Method: the rotated blur kernel's nonzero taps all equal 1/size and form a
digitized line. Per sample we pick the basis (identity / transpose /
unit shear, applied to the kernel) that minimizes PE streaming cost: taps
grouped by (sheared) kernel column give banded 0/1 weight matrices
contracted over image rows on the PE. The image is split into two fp8e4m3
planes (hi = fp8(x), lo = fp8(x - hi)) so each matmul runs in DoubleRow
perf mode (2 fp8 MACs/PE/cycle). Horizontal alignment and the shear are
baked into the host-side blob layout (a shear is linear in the row index),
so all device access patterns are static; sheared outputs are written to
DRAM in sheared form and unsheared on the host. Each slot's image windows
+ weight table arrive in one DMA; a post-compile pass drops back-to-back
reloads of identical PE weights; 1/size scaling happens in the PSUM->SBUF
copies, split between the Scalar and Vector engines.
"""

import math

import numpy as np
import ml_dtypes

MAXK = 32
H = W = 224
C = 3
WC = W * C                  # 672
PAD_LO = (MAXK - 1) // 2    # 15
PIMG_PAD = 480              # left zero margin (elems) of padded image row
PIMG_W = PIMG_PAD + WC + 600
WEDGE = 3 * 111             # extra sheared-output cols per 112-row block

FP8 = ml_dtypes.float8_e4m3


def dedupe_ldweights(nc):
    """Replace an InstLdweights whose weights AP is identical to the
    immediately-preceding one (per block) with an InstNoOp carrying its
    sync_info: the PE array still holds those weights, so the reload is
    pure Tensor-queue overhead (~158ns each)."""
    import concourse.mybir as mybir
    n = 0
    for fn in nc.m.functions:
        for blk in fn.blocks:
            prev_key = None
            new_insts = []
            for inst in blk.instructions:
                if isinstance(inst, mybir.InstLdweights):
                    key = (repr(inst.ins), repr(getattr(inst, "perf_mode", None)),
                           repr(getattr(inst, "is_transpose", None)))
                    if key == prev_key:
                        n += 1
                        inst = mybir.InstNoOp(
                            name=f"{inst.name}-ldwdedup",
                            engine=inst.engine,
                            ins=[], outs=[],
                            sync_info=inst.sync_info,
                        )
                    else:
                        prev_key = key
                new_insts.append(inst)
            blk.instructions = new_insts
    return n


# ---------------------------------------------------------------- host math
def rotate_nearest_np(img, rad):
    K = img.shape[0]
    cos, sin = np.cos(rad), np.sin(rad)
    coords = np.arange(K, dtype=np.float32)
    yy, xx = np.meshgrid(coords, coords, indexing="ij")
    e = np.float32(K - 1)
    x_off = (e - (cos * e - sin * e)) * 0.5
    y_off = (e - (sin * e + cos * e)) * 0.5
    sx = cos * xx - sin * yy + x_off
    sy = sin * xx + cos * yy + y_off
    ix = np.round(sx).astype(np.int32)
    iy = np.round(sy).astype(np.int32)
    valid = (ix >= 0) & (ix < K) & (iy >= 0) & (iy < K)
    g = img[np.clip(iy, 0, K - 1), np.clip(ix, 0, K - 1)]
    return np.where(valid, g, np.float32(0.0))


def _col_groups(ker):
    """Group nonzero taps of `ker` by column -> [(kx, klo, khi)], splitting
    any non-contiguous run."""
    ys, xs = np.nonzero(ker)
    groups = []
    for kx in np.unique(xs):
        run = np.sort(ys[xs == kx])
        start = prev = int(run[0])
        for v in run[1:]:
            v = int(v)
            if v == prev + 1:
                prev = v
            else:
                groups.append((int(kx), start, prev))
                start = prev = v
        groups.append((int(kx), start, prev))
    return groups


def _span(groups):
    if not groups:
        return 1
    kxs = [t[0] for t in groups]
    return max(kxs) - min(kxs) + 1


def shear_ker(ker):
    """ker'[ky, q] with q = kx - ky + 32 (unit shear; line angles in
    [0,90) have kx non-decreasing in ky, so this shrinks diagonal lines)."""
    K = ker.shape[0]
    wide = np.zeros((K, 2 * K + 1), ker.dtype)
    for ky in range(K):
        wide[ky, 32 - ky:32 - ky + K] = ker[ky]
    return wide


def sample_plan(tbl_ch0, amt_b, ang_b):
    """-> (scale, groups, transposed, sigma). groups are column-groups of
    the transformed kernel (column index q; true kx = q - 32*sigma +
    sigma*ky); basis minimizes span * streamed width."""
    rad = np.float32(ang_b * math.pi / 180.0)
    ker = rotate_nearest_np(tbl_ch0[amt_b], rad)
    ys, xs = np.nonzero(ker)
    if len(ys) == 0:
        return np.float32(0.0), [], False, 0
    scale = float(ker[ys[0], xs[0]])
    best = None
    for tr in (False, True):
        km = ker.T if tr else ker
        for sg in (0, 1):
            g = _col_groups(shear_ker(km) if sg else km)
            cost = _span(g) * (WC + WEDGE * sg)
            if best is None or cost < best[0]:
                best = (cost, g, tr, sg)
    return np.float32(scale), best[1], best[2], best[3]


def band_matrices(klo, khi):
    """w0 [128,112]: img rows 0..127 x out rows 0..111 (band r-y in
    [klo-15, khi-15]); w1: img rows 96..223 x out rows 112..223 (tile row
    r = img row 96+r, band r-y in [klo+1, khi+1]). Band clipping at the
    partition edges implements the vertical SAME padding."""
    r = np.arange(128)[:, None]
    y = np.arange(112)[None, :]
    d = r - y
    w0 = ((d >= klo - PAD_LO) & (d <= khi - PAD_LO)).astype(np.float32)
    w1 = ((d >= klo + 1) & (d <= khi + 1)).astype(np.float32)
    return w0, w1


def prepare_host(x, kernels_table, amt, angles, n_cores=8):
    B = x.shape[0]
    assert B % n_cores == 0
    slots = B // n_cores
    tbl_ch0 = np.ascontiguousarray(kernels_table[:, :, :, 0])

    scales = np.zeros(B, np.float32)
    groups = []
    transposed = np.zeros(B, bool)
    sigmas = np.zeros(B, np.int64)
    spans = np.zeros(B, np.int64)
    for b in range(B):
        s, g, tr, sg = sample_plan(tbl_ch0, int(amt[b]), int(angles[b]))
        scales[b] = s
        groups.append(g)
        transposed[b] = tr
        sigmas[b] = sg
        spans[b] = _span(g)

    # sigma must be uniform within a slot (SPMD). Round the sheared class
    # to a multiple of n_cores by moving the samples with the smallest
    # cost difference to the other basis, then balance on span per class.
    cost1 = np.array([spans[b] * (WC + WEDGE * sigmas[b]) for b in range(B)])
    cls = sigmas.copy()
    n1 = int((cls == 1).sum())
    rem = n1 % n_cores
    if rem:
        # re-plan candidates on the opposite basis and compute the penalty
        cand = np.where(cls == 1)[0]
        pen = []
        for b in cand:
            km = tbl_ch0[int(amt[b])]
            rad = np.float32(int(angles[b]) * math.pi / 180.0)
            ker = rotate_nearest_np(km, rad)
            g0 = _col_groups(ker)
            gt = _col_groups(ker.T)
            c0 = min(_span(g0) * WC, _span(gt) * WC)
            pen.append((c0 - cost1[b], b))
        pen.sort()
        for _, b in pen[:rem]:
            ker = rotate_nearest_np(tbl_ch0[int(amt[b])],
                                    np.float32(int(angles[b]) * math.pi / 180.0))
            g0 = _col_groups(ker)
            gt = _col_groups(ker.T)
            if _span(gt) < _span(g0):
                groups[b], transposed[b] = gt, True
            else:
                groups[b], transposed[b] = g0, False
            sigmas[b] = 0
            cls[b] = 0
            spans[b] = _span(groups[b])

    asg_rows = []
    row_sigma = []
    for sg in (0, 1):
        idx = np.where(cls == sg)[0]
        if len(idx) == 0:
            continue
        assert len(idx) % n_cores == 0
        order = idx[np.argsort(-spans[idx], kind="stable")]
        rows = order.reshape(len(idx) // n_cores, n_cores)
        for r in rows:
            asg_rows.append(r)
            row_sigma.append(sg)
    asg = np.stack(asg_rows)
    row_sigma = np.array(row_sigma)
    assert asg.shape == (slots, n_cores)

    # schedule: lightest slot first (its input DMA completes fastest, so
    # the PE starts early), a light slot last (small tail), heavy middle.
    slot_cost = np.array([max(1, spans[asg[j]].max()) * (WC + WEDGE * row_sigma[j])
                          for j in range(slots)])
    order = np.argsort(-slot_cost, kind="stable")   # heavy .. light
    sched = np.concatenate([[order[-2]], order[:-2], [order[-1]]])
    asg = asg[sched]
    row_sigma = row_sigma[sched]

    gmax = np.array([max(1, spans[asg[j]].max()) for j in range(slots)])
    wout = WC + WEDGE * row_sigma                    # result width per row block
    wprime = 3 * gmax + wout                         # moving window width
    wprime = ((wprime + 7) // 8) * 8
    blobw = 4 * wprime + 224 * gmax                  # 4 image planes + wt table
    col_base = np.concatenate([[0], np.cumsum(blobw)])[:-1]
    totbw = int(blobw.sum())
    out_base = np.concatenate([[0], np.cumsum(2 * 112 * wout)])[:-1]
    totout = int((2 * 112 * wout).sum())

    # fp8 hi/lo planes of the full batch
    x8hi = x.astype(FP8)
    xlo = x - x8hi.astype(np.float32)
    x8lo = xlo.astype(FP8)

    in_maps = []
    mapping = np.zeros((n_cores, slots), np.int64)
    for c in range(n_cores):
        blob = np.zeros((128, totbw), FP8)
        scl = np.zeros((128, slots), np.float32)
        for j in range(slots):
            b = int(asg[j, c])
            G = int(gmax[j])
            sg = int(row_sigma[j])
            Wp = int(wprime[j])
            base = int(col_base[j])
            mapping[c, j] = b
            scl[:, j] = scales[b]

            if transposed[b]:
                hi = np.ascontiguousarray(x8hi[b].transpose(1, 0, 2)).reshape(H, WC)
                lo = np.ascontiguousarray(x8lo[b].transpose(1, 0, 2)).reshape(H, WC)
            else:
                hi = x8hi[b].reshape(H, WC)
                lo = x8lo[b].reshape(H, WC)
            phi = np.zeros((H, PIMG_W), FP8)
            plo = np.zeros((H, PIMG_W), FP8)
            phi[:, PIMG_PAD:PIMG_PAD + WC] = hi
            plo[:, PIMG_PAD:PIMG_PAD + WC] = lo

            # group code cols q: true kx = q - 32*sg + sg*ky
            bk = min(t[0] for t in groups[b]) if groups[b] else 0
            # window row p of block hb covers pimg cols
            #   V0 + 3*sg*p + [0, Wp); out tile col u (psum col) holds
            #   out[R+r, w + 3*sg*r + u] with w = -WEDGE*sg.
            # matching: rhs col u' = u + 3*(q - bk) reads tap (q, ky) when
            #   V0 = PIMG_PAD - 45 + w + 3*(bk - 32*sg) + 3*sg*(S + 15 - R)
            for hb, (R, S) in enumerate(((0, 0), (112, 96))):
                V0 = (PIMG_PAD - 45 - WEDGE * sg + 3 * (bk - 32 * sg)
                      + 3 * sg * (S + 15 - R))
                assert 0 <= V0 and V0 + 3 * sg * 127 + Wp <= PIMG_W, \
                    (V0, sg, bk, Wp)
                rows = np.arange(128)
                cols = V0 + 3 * sg * rows
                src_rows = S + rows
                for pl, pimg in enumerate((phi, plo)):
                    dst = base + (2 * hb + pl) * Wp
                    win = np.zeros((128, Wp), FP8)
                    for p in range(128):
                        sr = src_rows[p]
                        if 0 <= sr < H:
                            win[p] = pimg[sr, cols[p]:cols[p] + Wp]
                    blob[:, dst:dst + Wp] = win

            # weight table: [G, 2(hb), 112] fp8, code col q = bk + g
            wtb = base + 4 * Wp
            wcols = np.zeros((128, G, 2, 112), np.float32)
            for q, klo, khi in groups[b]:
                i = q - bk
                assert 0 <= i < G, (b, q, bk, G)
                w0, w1 = band_matrices(klo, khi)
                wcols[:, i, 0, :] += w0
                wcols[:, i, 1, :] += w1
            blob[:, wtb:wtb + 224 * G] = np.ascontiguousarray(wcols).reshape(128, 224 * G).astype(FP8)
        in_maps.append({"ximg": blob, "scl": scl})

    meta = {
        "slots": slots,
        "gmax": [int(v) for v in gmax],
        "sigma": [int(v) for v in row_sigma],
        "wout": [int(v) for v in wout],
        "wprime": [int(v) for v in wprime],
        "blobw": [int(v) for v in blobw],
        "col_base": [int(v) for v in col_base],
        "out_base": [int(v) for v in out_base],
        "totbw": totbw,
        "totout": totout,
        "mapping": mapping,
        "transposed": transposed,
    }
    return meta, in_maps


def _chunks(wout):
    """Split a result width into <=512-col PSUM chunks (2 per row block)."""
    half = (wout + 1) // 2
    assert half <= 512
    return [(0, half), (half, wout - half)]


# ---------------------------------------------------------------- device IR
def build_program(meta):
    import concourse.bacc as bacc
    import concourse.mybir as mybir
    from concourse.tile import TileContext
    from bass_rust import VecI64Pair

    fp8 = mybir.dt.float8e4
    slots = meta["slots"]

    nc = bacc.Bacc("TRN2")
    ximg = nc.dram_tensor("ximg", [128, meta["totbw"]], fp8, kind="ExternalInput")
    scl = nc.dram_tensor("scl", [128, slots], mybir.dt.float32,
                         kind="ExternalInput")
    out = nc.dram_tensor("out", [1, meta["totout"]], mybir.dt.float16,
                         kind="ExternalOutput")

    def strided(tile, dims, offset):
        ap = tile[:, 0:1].copy()
        ap.ap = VecI64Pair(dims)
        ap.offset = offset
        return ap

    with TileContext(nc) as tc:
        with tc.tile_pool(name="const", bufs=1) as cpool, \
             tc.tile_pool(name="img", bufs=4) as ipool, \
             tc.tile_pool(name="res", bufs=4) as rpool, \
             tc.tile_pool(name="ps0", bufs=2, space="PSUM") as pp00, \
             tc.tile_pool(name="ps1", bufs=2, space="PSUM") as pp01, \
             tc.tile_pool(name="ps2", bufs=2, space="PSUM") as pp10, \
             tc.tile_pool(name="ps3", bufs=2, space="PSUM") as pp11:
            st = cpool.tile([128, slots], mybir.dt.float32)
            nc.scalar.dma_start(out=st, in_=scl[:, :])

            pools = [[pp00, pp01], [pp10, pp11]]
            for j in range(slots):
                G = meta["gmax"][j]
                WO = meta["wout"][j]
                Wp = meta["wprime"][j]
                BW = meta["blobw"][j]
                base = meta["col_base"][j]
                obase = meta["out_base"][j]
                ch = _chunks(WO)
                blob = ipool.tile([128, BW], fp8, tag="blob", name="blob")
                wtb = 4 * Wp
                nc.sync.dma_start(out=blob, in_=ximg[:, base:base + BW])

                psums = [[pools[hb][wh].tile([112, ch[wh][1]], mybir.dt.float32,
                                             tag=f"ps{hb}{wh}", name=f"ps{hb}{wh}")
                          for wh in (0, 1)] for hb in (0, 1)]
                rt = rpool.tile([112, 2 * WO], mybir.dt.float16, tag="rt",
                                name="rt")
                sc = st[0:112, j:j + 1]
                for hb in (0, 1):
                    for g in range(G):
                        # same band matrix for both fp8 planes (hi, lo)
                        lhs = strided(blob, [[BW, 128], [0, 2], [1, 112]],
                                      wtb + 224 * g + 112 * hb)
                        for wh in (0, 1):
                            # planes (hi, lo) of window hb at column shift 3g
                            rhs = strided(
                                blob, [[BW, 128], [Wp, 2], [1, ch[wh][1]]],
                                2 * hb * Wp + 3 * g + ch[wh][0])
                            nc.tensor.matmul(
                                psums[hb][wh], lhsT=lhs, rhs=rhs,
                                start=(g == 0), stop=(g == G - 1),
                                perf_mode=mybir.MatmulPerfMode.DoubleRow)
                    dst0 = rt[:, hb * WO + ch[0][0]:hb * WO + ch[0][0] + ch[0][1]]
                    dst1 = rt[:, hb * WO + ch[1][0]:hb * WO + ch[1][0] + ch[1][1]]
                    nc.scalar.activation(
                        out=dst0, in_=psums[hb][0],
                        func=mybir.ActivationFunctionType.Copy, scale=sc)
                    nc.vector.tensor_scalar_mul(out=dst1, in0=psums[hb][1],
                                                scalar1=sc)
                src = strided(rt, [[2 * WO, 112], [WO, 2], [1, WO]], 0)
                dst = out[0, 0:1].copy()
                dst.ap = VecI64Pair([[WO, 112], [112 * WO, 2], [1, WO]])
                dst.offset = obase
                # final slots drain on the otherwise-idle sync queue so the
                # end-of-kernel DMA flush runs on two queues in parallel
                eng = nc.sync if j >= slots - 2 else nc.gpsimd
                eng.dma_start(out=dst, in_=src)
    return nc


def run_cores(meta, in_maps, trace=False):
    from concourse.bass_utils import run_bass_kernel_spmd

    nc = build_program(meta)
    nc.compile()
    dedupe_ldweights(nc)
    res = run_bass_kernel_spmd(nc, in_maps, core_ids=list(range(len(in_maps))),
                               trace=trace)
    return res


def unshard(meta, results):
    B = meta["mapping"].size
    out = np.zeros((B, H, W, C), np.float32)
    for c, r in enumerate(results):
        o = np.asarray(r["out"], np.float32).reshape(-1)
        for j in range(meta["slots"]):
            b = meta["mapping"][c, j]
            WO = meta["wout"][j]
            sg = meta["sigma"][j]
            t = o[meta["out_base"][j]:meta["out_base"][j] + 2 * 112 * WO]
            t = t.reshape(2, 112, WO)
            img = np.zeros((H, WC), np.float32)
            if sg == 0:
                img[0:112] = t[0, :, 0:WC]
                img[112:224] = t[1, :, 0:WC]
            else:
                for r_ in range(112):
                    u = 3 * (111 - r_)
                    img[r_] = t[0, r_, u:u + WC]
                    img[112 + r_] = t[1, r_, u:u + WC]
            img = img.reshape(H, W, C)
            if meta["transposed"][b]:
                img = img.transpose(1, 0, 2)
            out[b] = img
    return out


def kernel(x, kernels_table, amt, angles):
    x = np.asarray(x, np.float32)
    kernels_table = np.asarray(kernels_table, np.float32)
    amt = np.asarray(amt)
    angles = np.asarray(angles)
    meta, in_maps = prepare_host(x, kernels_table, amt, angles)
    res = run_cores(meta, in_maps)
    return unshard(meta, res.results)
